# revision 1
# baseline (speedup 1.0000x reference)
"""Trainium2 Bass kernel for two-level segment mean (tokens->mentions->entities).

Math: the reference computes
    mentions[m] = (1/max(cnt_m[m],1)) * sum_{t: token2mention[t]=m} enc_seq[t]
    entities[e] = (1/max(cnt_e[e],1)) * sum_{m: mention2entity[m]=e} mentions[m]
which collapses to a single weighted segment-sum over tokens:
    entities[e] = sum_{t: ent(t)=e} enc_seq[t] / (cnt_m[men(t)] * max(cnt_e[e],1))
(empty mentions contribute zero vectors; cnt_e counts mentions incl. empty ones).

Sharding: entities are packed into tiles of entity slots, balanced by token
count (LPT).  Each token belongs to exactly one entity and hence one core ->
pure data parallel, no collectives.  On device each tile's tokens stream
through the TensorEngine as 128-token chunks: a one-hot selection matrix
S[t, slot] = (slot == ent_slot(t)) selects rows, psum += S^T @ X.

Default mode "e3m4p": X is fp8 e3m4 carrying enc_seq*(4/cnt_m) (the mention
mean; ~1.4e-2 rel err, inside the 2e-2 gate at 1 byte/element); the entity
mean 1/cnt_e is per-output-slot and lands in the final per-partition-scalar
psum->SBUF copy.  Entities live in PAIRS of 64-slot tiles: the pair's
matmuls go to PE column groups (tile_position (0,0)/(0,64)) accumulating
into the two partition halves of one PSUM tile, and its one-hots are built
by a single batched DVE is_equal (stride-0 broadcast APs) over 64 columns
only.  Output is written bf16 and upcast on host.  Older modes (bf16,
e3m4s, fp16x2 hi/lo with ~8e-8 err, fp32) kept for reference/fallback.
"""

import sys
import heapq

import numpy as np

for _p in ("/opt/trn_rl_repo",):
    if _p not in sys.path:
        sys.path.insert(0, _p)

P = 128
NCORES = 8
S_HI = np.float32(128.0)      # 2**7
S_LO = np.float32(2048.0)     # 2**11


def _pack_entities(cnt_te, n_tiles, cap=P):
    """LPT-pack entities into n_tiles tiles of <=cap slots, balancing token
    load.  Returns (tile_of_ent, slot_of_ent, C) where C = max 128-token
    chunks per tile."""
    E = cnt_te.shape[0]
    order_e = np.argsort(-cnt_te, kind="stable")
    tile_of_ent = np.empty(E, np.int32)
    slot_of_ent = np.empty(E, np.int32)
    h = [(0, 0, i) for i in range(n_tiles)]
    heapq.heapify(h)
    for ent in order_e:
        c = int(cnt_te[ent])
        popped = []
        while True:
            load, sl, t = heapq.heappop(h)
            if sl < cap:
                break
            popped.append((load, sl, t))
        for p in popped:
            heapq.heappush(h, p)
        tile_of_ent[ent] = t
        slot_of_ent[ent] = sl
        heapq.heappush(h, (load + c, sl + 1, t))
    loads = np.bincount(tile_of_ent, weights=cnt_te.astype(np.float64),
                        minlength=n_tiles)
    C = max(1, int(np.ceil(loads.max() / P)))
    return tile_of_ent, slot_of_ent, C


def _build_program(KPT, C, D, repeat=1, mode="e3m4p"):
    """Build the SPMD Bass program (identical for all cores)."""
    import concourse.bacc as bacc
    import concourse.mybir as mybir
    import concourse.tile as tile

    NCH = KPT * C
    f32 = mybir.dt.float32
    f16 = mybir.dt.float16
    bf16 = mybir.dt.bfloat16

    nc = bacc.Bacc("TRN2", target_bir_lowering=False, debug=False,
                   num_devices=NCORES)
    if mode == "bf16":
        x_d = nc.dram_tensor("x", [P, NCH * D], bf16, kind="ExternalInput")
        el_d = nc.dram_tensor("el", [P, NCH], f32, kind="ExternalInput")
    elif mode == "e3m4s":
        x_d = nc.dram_tensor("x", [P, NCH * D], mybir.dt.float8e3,
                             kind="ExternalInput")
        el_d = nc.dram_tensor("el", [P, NCH], bf16, kind="ExternalInput")
        ws_d = nc.dram_tensor("ws", [P, KPT], f32, kind="ExternalInput")
        io_d = nc.dram_tensor("io", [P, P], bf16, kind="ExternalInput")
    elif mode == "e3m4p":
        # KPT = pairs of 64-slot tiles per core, C = 128-token chunks per tile
        x_d = nc.dram_tensor("x", [P, KPT * C * 2 * D], mybir.dt.float8e3,
                             kind="ExternalInput")
        el_d = nc.dram_tensor("el", [P, KPT * C * 2], bf16,
                              kind="ExternalInput")
        ws_d = nc.dram_tensor("ws", [P, KPT], f32, kind="ExternalInput")
        io_d = nc.dram_tensor("io", [P, 64], bf16, kind="ExternalInput")
    elif mode == "e3m4":
        x_d = nc.dram_tensor("x", [P, NCH * D], mybir.dt.float8e3,
                             kind="ExternalInput")
        el_d = nc.dram_tensor("el", [P, NCH], f32, kind="ExternalInput")
        pw_d = nc.dram_tensor("pw", [P, NCH], f32, kind="ExternalInput")
    elif mode == "fp16x2":
        x_d = nc.dram_tensor("x", [P, NCH * 2 * D], f16, kind="ExternalInput")
        el_d = nc.dram_tensor("el", [P, NCH], f32, kind="ExternalInput")
    else:
        x_d = nc.dram_tensor("x", [P, NCH * D], f32, kind="ExternalInput")
        el_d = nc.dram_tensor("el", [P, NCH], f32, kind="ExternalInput")
        rw_d = nc.dram_tensor("rw", [P, NCH], f32, kind="ExternalInput")
    out_dt = bf16 if mode in ("e3m4s", "e3m4p") else f32
    if mode == "e3m4p":
        # partition-major: contiguous 3KB-per-partition out-DMA runs
        out_d = nc.dram_tensor("out", [P, KPT * D], out_dt,
                               kind="ExternalOutput")
    else:
        out_d = nc.dram_tensor("out", [KPT * P, D], out_dt,
                               kind="ExternalOutput")

    with tile.TileContext(nc) as tc:
        def body_plane(x_dt, with_pw):
            """Single-plane pipeline: S one-hot (optionally scaled by a
            per-token power of two) matmul'd against the x plane."""
            GB = 8  # entity tiles per x-DMA / per out-DMA
            with (
                tc.tile_pool(name="const", bufs=1) as const,
                tc.tile_pool(name="x", bufs=2) as xpool,
                tc.tile_pool(name="s", bufs=8) as spool,
                tc.tile_pool(name="psum", bufs=4, space="PSUM") as ppool,
                tc.tile_pool(name="o", bufs=3) as opool,
            ):
                iota_t = const.tile([P, P], f32)
                nc.gpsimd.iota(iota_t[:], [[1, P]], base=0, channel_multiplier=0,
                               allow_small_or_imprecise_dtypes=True)
                el_sb = const.tile([P, NCH], f32)
                nc.sync.dma_start(out=el_sb[:], in_=el_d[:, :])
                if with_pw:
                    pw_sb = const.tile([P, NCH], f32)
                    nc.sync.dma_start(out=pw_sb[:], in_=pw_d[:, :])

                for jg in range(0, KPT, GB):
                    gn = min(GB, KPT - jg)
                    xt = xpool.tile([P, GB * C * D], x_dt)
                    nc.sync.dma_start(
                        out=xt[:, :gn * C * D],
                        in_=x_d[:, jg * C * D:(jg + gn) * C * D])
                    og = opool.tile([P, GB * D], f32, tag="og")
                    for g in range(gn):
                        j = jg + g
                        ps = ppool.tile([P, D], f32, tag="ps")
                        for i in range(C):
                            q = j * C + i
                            s = spool.tile([P, P], bf16)
                            eng = nc.vector
                            if with_pw:
                                eng.tensor_scalar(
                                    out=s[:], in0=iota_t[:],
                                    scalar1=el_sb[:, q:q + 1],
                                    scalar2=pw_sb[:, q:q + 1],
                                    op0=mybir.AluOpType.is_equal,
                                    op1=mybir.AluOpType.mult)
                            else:
                                eng.tensor_scalar(
                                    out=s[:], in0=iota_t[:],
                                    scalar1=el_sb[:, q:q + 1], scalar2=None,
                                    op0=mybir.AluOpType.is_equal)
                            base = (g * C + i) * D
                            nc.tensor.matmul(out=ps[:], lhsT=s[:],
                                             rhs=xt[:, base:base + D],
                                             start=(i == 0), stop=(i == C - 1))
                        nc.vector.tensor_copy(out=og[:, g * D:(g + 1) * D],
                                              in_=ps[:])
                    nc.sync.dma_start(
                        out=out_d[jg * P:(jg + gn) * P, :].rearrange(
                            "(g p) d -> p g d", p=P),
                        in_=og[:, :gn * D].rearrange("p (g d) -> p g d", g=gn))

        def body_bf16():
            body_plane(bf16, with_pw=False)

        def body_e3m4():
            body_plane(mybir.dt.float8e3, with_pw=True)

        def body_e3m4s():
            """e3m4 plane carrying x/cnt_m; S is a pure one-hot built in ONE
            batched DVE op per entity tile (broadcast is_equal); the per-slot
            1/cnt_e lands in the ACT-engine scaled copy of the psum."""
            from concourse.bass import broadcast_tensor_aps
            f83 = mybir.dt.float8e3
            GB = 8
            with (
                tc.tile_pool(name="const", bufs=2) as const,
                tc.tile_pool(name="x", bufs=2) as xpool,
                tc.tile_pool(name="s", bufs=3) as spool,
                tc.tile_pool(name="psum", bufs=4, space="PSUM") as ppool,
                tc.tile_pool(name="o", bufs=3) as opool,
            ):
                iota_t = const.tile([P, P], bf16)
                nc.sync.dma_start(out=iota_t[:], in_=io_d[:, :])
                el_sb = const.tile([P, NCH], bf16)
                nc.sync.dma_start(out=el_sb[:], in_=el_d[:, :])
                ws_sb = const.tile([P, KPT], f32)
                nc.sync.dma_start(out=ws_sb[:], in_=ws_d[:, :])

                for jg in range(0, KPT, GB):
                    gn = min(GB, KPT - jg)
                    xt = xpool.tile([P, GB * C * D], f83)
                    nc.sync.dma_start(
                        out=xt[:, :gn * C * D],
                        in_=x_d[:, jg * C * D:(jg + gn) * C * D])
                    og = opool.tile([P, GB * D], out_dt, tag="og")
                    for g in range(gn):
                        j = jg + g
                        sb_t = spool.tile([P, C * P], bf16)
                        iota_3d = iota_t[:, :].rearrange("p (a c) -> p a c", a=1)
                        el_3d = el_sb[:, j * C:(j + 1) * C].rearrange(
                            "p (a c) -> p a c", c=1)
                        iota_bc, el_bc = broadcast_tensor_aps(iota_3d, el_3d)
                        nc.vector.tensor_tensor(
                            out=sb_t[:, :].rearrange("p (a c) -> p a c", c=P),
                            in0=iota_bc, in1=el_bc,
                            op=mybir.AluOpType.is_equal)
                        ps = ppool.tile([P, D], f32, tag="ps")
                        for i in range(C):
                            base = (g * C + i) * D
                            nc.tensor.matmul(out=ps[:],
                                             lhsT=sb_t[:, i * P:(i + 1) * P],
                                             rhs=xt[:, base:base + D],
                                             start=(i == 0), stop=(i == C - 1))
                        nc.vector.tensor_scalar(
                            out=og[:, g * D:(g + 1) * D], in0=ps[:],
                            scalar1=ws_sb[:, j:j + 1], scalar2=None,
                            op0=mybir.AluOpType.mult)
                    nc.sync.dma_start(
                        out=out_d[jg * P:(jg + gn) * P, :].rearrange(
                            "(g p) d -> p g d", p=P),
                        in_=og[:, :gn * D].rearrange("p (g d) -> p g d", g=gn))

        def body_e3m4p():
            """Paired 64-slot tiles: the two halves of each pair run as
            column-tiled concurrent matmuls (tile_position (0,0)/(0,64)) into
            the two partition halves of one PSUM tile, halving PE stream time
            and halving the one-hot comparisons on DVE."""
            from concourse.bass import broadcast_tensor_aps
            f83 = mybir.dt.float8e3
            U = KPT
            GB = 8
            with (
                tc.tile_pool(name="const", bufs=2) as const,
                tc.tile_pool(name="x", bufs=4) as xpool,
                tc.tile_pool(name="s", bufs=3) as spool,
                tc.tile_pool(name="psum", bufs=8, space="PSUM") as ppool,
                tc.tile_pool(name="o", bufs=4) as opool,
            ):
                iota_t = const.tile([P, 64], bf16)
                nc.sync.dma_start(out=iota_t[:], in_=io_d[:, :])
                el_sb = const.tile([P, U * C * 2], bf16)
                nc.sync.dma_start(out=el_sb[:], in_=el_d[:, :])
                ws_sb = const.tile([P, U], f32)
                nc.sync.dma_start(out=ws_sb[:], in_=ws_d[:, :])

                for jg in range(0, U, GB):
                    gn = min(GB, U - jg)
                    xt = xpool.tile([P, GB * C * 2 * D], f83)
                    nc.sync.dma_start(
                        out=xt[:, :gn * C * 2 * D],
                        in_=x_d[:, jg * C * 2 * D:(jg + gn) * C * 2 * D])
                    og = opool.tile([P, GB * D], out_dt, tag="og")
                    # one batched one-hot build for the whole group (8 pairs)
                    sg = spool.tile([P, GB * C * 2 * 64], f83, tag="sg")
                    i3 = iota_t[:, :].rearrange("p (a c) -> p a c", a=1)
                    e3 = el_sb[:, jg * C * 2:(jg + gn) * C * 2].rearrange(
                        "p (a c) -> p a c", c=1)
                    ib, eb = broadcast_tensor_aps(i3, e3)
                    nc.vector.tensor_tensor(
                        out=sg[:, :gn * C * 2 * 64].rearrange(
                            "p (a c) -> p a c", c=64),
                        in0=ib, in1=eb, op=mybir.AluOpType.is_equal)
                    for g in range(gn):
                        u = jg + g
                        ps = ppool.tile([P, D], f32, tag="ps")
                        for i in range(C):
                            for h in range(2):
                                sbase = (g * C + i) * 2 * 64 + h * 64
                                xbase = ((g * C + i) * 2 + h) * D
                                nc.tensor.matmul(
                                    out=ps[h * 64:(h + 1) * 64, :],
                                    lhsT=sg[:, sbase:sbase + 64],
                                    rhs=xt[:, xbase:xbase + D],
                                    start=(i == 0), stop=(i == C - 1),
                                    tile_position=(0, h * 64))
                        nc.vector.tensor_scalar(
                            out=og[:, g * D:(g + 1) * D], in0=ps[:],
                            scalar1=ws_sb[:, u:u + 1], scalar2=None,
                            op0=mybir.AluOpType.mult)
                    nc.sync.dma_start(
                        out=out_d[:, jg * D:(jg + gn) * D],
                        in_=og[:, :gn * D])

        def body_fp16():
            GB = 4  # entity tiles per x-DMA (8.25 MB) / per out-DMA
            with (
                tc.tile_pool(name="const", bufs=1) as const,
                tc.tile_pool(name="x", bufs=2) as xpool,
                tc.tile_pool(name="s", bufs=8) as spool,
                tc.tile_pool(name="psum", bufs=3, space="PSUM") as ppool,
                tc.tile_pool(name="o", bufs=3) as opool,
            ):
                iota_t = const.tile([P, P], f32)
                nc.gpsimd.iota(iota_t[:], [[1, P]], base=0, channel_multiplier=0,
                               allow_small_or_imprecise_dtypes=True)
                el_sb = const.tile([P, NCH], f32)
                nc.sync.dma_start(out=el_sb[:], in_=el_d[:, :])

                for jg in range(0, KPT, GB):
                    gn = min(GB, KPT - jg)
                    xt = xpool.tile([P, GB * C * 2 * D], f16)
                    nc.sync.dma_start(
                        out=xt[:, :gn * C * 2 * D],
                        in_=x_d[:, jg * C * 2 * D:(jg + gn) * C * 2 * D])
                    og = opool.tile([P, GB * D], f32, tag="og")
                    for g in range(gn):
                        j = jg + g
                        ph = ppool.tile([P, D], f32, tag="ph")
                        pl = ppool.tile([P, D], f32, tag="pl")
                        for i in range(C):
                            q = j * C + i
                            s = spool.tile([P, P], f16)
                            nc.vector.tensor_scalar(
                                out=s[:], in0=iota_t[:],
                                scalar1=el_sb[:, q:q + 1], scalar2=None,
                                op0=mybir.AluOpType.is_equal)
                            base = (g * C + i) * 2 * D
                            nc.tensor.matmul(out=ph[:], lhsT=s[:],
                                             rhs=xt[:, base:base + D],
                                             start=(i == 0), stop=(i == C - 1))
                            nc.tensor.matmul(out=pl[:], lhsT=s[:],
                                             rhs=xt[:, base + D:base + 2 * D],
                                             start=(i == 0), stop=(i == C - 1))
                        oa = opool.tile([P, D], f32, tag="oa")
                        nc.vector.tensor_scalar(
                            out=oa[:], in0=pl[:], scalar1=float(1.0 / S_LO),
                            scalar2=None, op0=mybir.AluOpType.mult)
                        ob = opool.tile([P, D], f32, tag="ob")
                        nc.vector.tensor_tensor(
                            out=ob[:], in0=oa[:], in1=ph[:],
                            op=mybir.AluOpType.add)
                        nc.vector.tensor_scalar(
                            out=og[:, g * D:(g + 1) * D], in0=ob[:],
                            scalar1=float(1.0 / S_HI),
                            scalar2=None, op0=mybir.AluOpType.mult)
                    nc.sync.dma_start(
                        out=out_d[jg * P:(jg + gn) * P, :].rearrange(
                            "(g p) d -> p g d", p=P),
                        in_=og[:, :gn * D].rearrange("p (g d) -> p g d", g=gn))

        def body_fp32():
            with (
                tc.tile_pool(name="const", bufs=1) as const,
                tc.tile_pool(name="x", bufs=3) as xpool,
                tc.tile_pool(name="s", bufs=8) as spool,
                tc.tile_pool(name="psum", bufs=4, space="PSUM") as ppool,
                tc.tile_pool(name="o", bufs=4) as opool,
            ):
                iota_t = const.tile([P, P], f32)
                nc.gpsimd.iota(iota_t[:], [[1, P]], base=0, channel_multiplier=0,
                               allow_small_or_imprecise_dtypes=True)
                el_sb = const.tile([P, NCH], f32)
                nc.sync.dma_start(out=el_sb[:], in_=el_d[:, :])
                rw_sb = const.tile([P, NCH], f32)
                nc.sync.dma_start(out=rw_sb[:], in_=rw_d[:, :])

                for j in range(KPT):
                    xt = xpool.tile([P, C * D], f32)
                    nc.sync.dma_start(out=xt[:],
                                      in_=x_d[:, j * C * D:(j + 1) * C * D])
                    ps = ppool.tile([P, D], f32)
                    for i in range(C):
                        q = j * C + i
                        s = spool.tile([P, P], f32)
                        nc.vector.tensor_scalar(
                            out=s[:], in0=iota_t[:],
                            scalar1=el_sb[:, q:q + 1], scalar2=rw_sb[:, q:q + 1],
                            op0=mybir.AluOpType.is_equal,
                            op1=mybir.AluOpType.mult)
                        nc.tensor.matmul(out=ps[:], lhsT=s[:],
                                         rhs=xt[:, i * D:(i + 1) * D],
                                         start=(i == 0), stop=(i == C - 1))
                    ot = opool.tile([P, D], f32)
                    nc.vector.tensor_copy(out=ot[:], in_=ps[:])
                    nc.sync.dma_start(out=out_d[j * P:(j + 1) * P, :], in_=ot[:])

        body = {"bf16": body_bf16, "e3m4": body_e3m4, "e3m4s": body_e3m4s,
                "e3m4p": body_e3m4p, "fp16x2": body_fp16,
                "fp32": body_fp32}[mode]
        if repeat == 1:
            body()
        else:
            with tc.For_i(0, repeat, 1):
                body()

    nc.compile()
    return nc


def _prepare(enc_seq, token2mention, mention2entity, num_mentions, num_entities,
             mode="e3m4p"):
    """Host-side shard/stage: returns (in_maps, meta) for the 8 cores."""
    enc_seq = np.ascontiguousarray(np.asarray(enc_seq, dtype=np.float32))
    t2m = np.asarray(token2mention).astype(np.int64, copy=False)
    m2e = np.asarray(mention2entity).astype(np.int64, copy=False)
    M = int(num_mentions)
    E = int(num_entities)
    T, D = enc_seq.shape

    e_of_tok = m2e[t2m]                              # [T] entity of each token
    cnt_m = np.bincount(t2m, minlength=M)            # tokens per mention
    cnt_e = np.bincount(m2e, minlength=E)            # mentions per entity
    cnt_te = np.bincount(e_of_tok, minlength=E)      # tokens per entity

    if mode == "e3m4p":
        import ml_dtypes
        SC = np.float32(4.0)
        # pairs of 64-slot tiles; 2U tiles per core
        U = int(np.ceil(np.ceil(E / 64) / (2 * NCORES)))
        n_tiles = NCORES * U * 2
        tile_of_ent, slot_of_ent, C = _pack_entities(cnt_te, n_tiles, cap=64)

        tile_of_tok = tile_of_ent[e_of_tok]
        order = np.argsort(tile_of_tok, kind="stable")
        tile_sorted = tile_of_tok[order]
        tile_counts = np.bincount(tile_of_tok, minlength=n_tiles)
        tile_start = np.concatenate([[0], np.cumsum(tile_counts[:-1])])
        pos_sorted = np.arange(T, dtype=np.int64) - tile_start[tile_sorted]
        i_sorted = pos_sorted // P
        p_sorted = pos_sorted % P
        core_s = tile_sorted // (2 * U)
        l_s = tile_sorted % (2 * U)
        q2_sorted = (l_s // 2 * C + i_sorted) * 2 + (l_s % 2)
        core_tok = np.empty(T, np.int64)
        p_tok = np.empty(T, np.int64)
        q_tok = np.empty(T, np.int64)
        core_tok[order] = core_s
        p_tok[order] = p_sorted
        q_tok[order] = q2_sorted

        NQ = U * C * 2
        X = np.zeros((NCORES, P, NQ, D), ml_dtypes.float8_e3m4)
        wm = (SC / np.maximum(cnt_m, 1).astype(np.float32))[t2m]
        BS = 1 << 18
        for s0 in range(0, T, BS):
            s1 = min(s0 + BS, T)
            v = enc_seq[s0:s1] * wm[s0:s1, None]
            np.clip(v, -15.5, 15.5, out=v)
            c, p, q = core_tok[s0:s1], p_tok[s0:s1], q_tok[s0:s1]
            X[c, p, q] = v.astype(ml_dtypes.float8_e3m4)
        el = np.full((NCORES, P, NQ), -1.0, ml_dtypes.bfloat16)
        el[core_tok, p_tok, q_tok] = (slot_of_ent[e_of_tok]
                                      .astype(ml_dtypes.bfloat16))
        ws = np.zeros((NCORES, P, U), np.float32)
        core_e = (tile_of_ent // (2 * U)).astype(np.int64)
        l_e = tile_of_ent % (2 * U)
        u_e = (l_e // 2).astype(np.int64)
        h_e = (l_e % 2).astype(np.int64)
        ws[core_e, h_e * 64 + slot_of_ent, u_e] = (
            1.0 / np.maximum(cnt_e, 1) / SC).astype(np.float32)
        io = np.broadcast_to(np.arange(64, dtype=np.float32),
                             (P, 64)).astype(ml_dtypes.bfloat16)
        in_maps = []
        for c in range(NCORES):
            in_maps.append({
                "x": X[c].reshape(P, NQ * D),
                "el": el[c],
                "ws": ws[c],
                "io": np.ascontiguousarray(io),
            })
        meta = dict(KPT=U, C=C, D=D, E=E, mode=mode,
                    core_e=core_e, u_e=u_e,
                    hs_e=h_e * 64 + slot_of_ent)
        return in_maps, meta

    # tiles of <=128 entity slots, token-count balanced; KPT tiles per core
    KPT = int(np.ceil(np.ceil(E / P) / NCORES))
    n_tiles = NCORES * KPT
    tile_of_ent, slot_of_ent, C = _pack_entities(cnt_te, n_tiles)
    NCH = KPT * C

    # destination row for each token: tiles are laid out back to back with
    # C*P rows each; within a tile, tokens in stable order
    tile_of_tok = tile_of_ent[e_of_tok]
    order = np.argsort(tile_of_tok, kind="stable")
    tile_sorted = tile_of_tok[order]
    tile_counts = np.bincount(tile_of_tok, minlength=n_tiles)
    tile_start = np.concatenate([[0], np.cumsum(tile_counts[:-1])])
    pos_sorted = np.arange(T, dtype=np.int64) - tile_start[tile_sorted]
    dst_sorted = tile_sorted.astype(np.int64) * (C * P) + pos_sorted
    dst_row = np.empty(T, np.int64)
    dst_row[order] = dst_sorted                       # per-token dest row

    rows_per_core = KPT * C * P
    core_tok = (dst_row // rows_per_core).astype(np.int64)
    lr = dst_row % rows_per_core
    q_tok = (lr // P).astype(np.int64)                # chunk within core
    p_tok = (lr % P).astype(np.int64)                 # partition

    # total per-token weight: 1/cnt_m (mention mean) * 1/max(cnt_e,1)
    # (entity mean, folded in so no divide is needed on device)
    w_tok = ((1.0 / np.maximum(cnt_m, 1))[t2m]
             * (1.0 / np.maximum(cnt_e, 1))[e_of_tok]).astype(np.float32)

    in_maps = []
    if mode == "bf16":
        import ml_dtypes
        X = np.zeros((NCORES, P, NCH, D), ml_dtypes.bfloat16)
        BS = 1 << 18
        for s0 in range(0, T, BS):
            s1 = min(s0 + BS, T)
            v = enc_seq[s0:s1] * w_tok[s0:s1, None]
            c, p, q = core_tok[s0:s1], p_tok[s0:s1], q_tok[s0:s1]
            X[c, p, q] = v.astype(ml_dtypes.bfloat16)
        el = np.full((NCORES, P, NCH), -1.0, np.float32)
        el[core_tok, p_tok, q_tok] = slot_of_ent[e_of_tok].astype(np.float32)
        for c in range(NCORES):
            in_maps.append({
                "x": X[c].reshape(P, NCH * D),
                "el": el[c],
            })
    elif mode == "e3m4s":
        import ml_dtypes
        SC = np.float32(4.0)
        X = np.zeros((NCORES, P, NCH, D), ml_dtypes.float8_e3m4)
        wm = (SC / np.maximum(cnt_m, 1).astype(np.float32))[t2m]
        BS = 1 << 18
        for s0 in range(0, T, BS):
            s1 = min(s0 + BS, T)
            v = enc_seq[s0:s1] * wm[s0:s1, None]
            np.clip(v, -15.5, 15.5, out=v)
            c, p, q = core_tok[s0:s1], p_tok[s0:s1], q_tok[s0:s1]
            X[c, p, q] = v.astype(ml_dtypes.float8_e3m4)
        el = np.full((NCORES, P, NCH), -1.0, ml_dtypes.bfloat16)
        el[core_tok, p_tok, q_tok] = slot_of_ent[e_of_tok].astype(
            ml_dtypes.bfloat16)
        ws = np.zeros((NCORES, P, KPT), np.float32)
        ents = np.arange(E)
        ws[tile_of_ent // KPT, slot_of_ent, tile_of_ent % KPT] = (
            1.0 / np.maximum(cnt_e[ents], 1) / SC).astype(np.float32)
        io = np.broadcast_to(np.arange(P, dtype=np.float32),
                             (P, P)).astype(ml_dtypes.bfloat16)
        for c in range(NCORES):
            in_maps.append({
                "x": X[c].reshape(P, NCH * D),
                "el": el[c],
                "ws": ws[c],
                "io": np.ascontiguousarray(io),
            })
    elif mode == "e3m4":
        import ml_dtypes
        # per-token power-of-two split: w = m_t * 2^e_t with m_t in
        # [0.71, 1.41]; the e3m4 plane holds x*m_t*2 (O(1) dynamic range,
        # clipped to +-15.5) and S carries the exact 2^e_t/2 factor.
        e_exp = np.round(np.log2(w_tok)).astype(np.int32)
        m_t = (w_tok * np.exp2(-e_exp.astype(np.float32)))
        X = np.zeros((NCORES, P, NCH, D), ml_dtypes.float8_e3m4)
        BS = 1 << 18
        for s0 in range(0, T, BS):
            s1 = min(s0 + BS, T)
            v = enc_seq[s0:s1] * (2.0 * m_t[s0:s1, None])
            np.clip(v, -15.5, 15.5, out=v)
            c, p, q = core_tok[s0:s1], p_tok[s0:s1], q_tok[s0:s1]
            X[c, p, q] = v.astype(ml_dtypes.float8_e3m4)
        el = np.full((NCORES, P, NCH), -1.0, np.float32)
        el[core_tok, p_tok, q_tok] = slot_of_ent[e_of_tok].astype(np.float32)
        pw = np.zeros((NCORES, P, NCH), np.float32)
        pw[core_tok, p_tok, q_tok] = np.exp2(e_exp.astype(np.float32) - 1.0)
        for c in range(NCORES):
            in_maps.append({
                "x": X[c].reshape(P, NCH * D),
                "el": el[c],
                "pw": pw[c],
            })
    elif mode == "fp16x2":
        X = np.zeros((NCORES, P, NCH, 2, D), np.float16)
        # block the hi/lo computation to bound temp memory
        BS = 1 << 18
        for s0 in range(0, T, BS):
            s1 = min(s0 + BS, T)
            v = enc_seq[s0:s1] * (w_tok[s0:s1, None] * S_HI)
            hi = v.astype(np.float16)
            lo = ((v - hi.astype(np.float32)) * S_LO).astype(np.float16)
            c, p, q = core_tok[s0:s1], p_tok[s0:s1], q_tok[s0:s1]
            X[c, p, q, 0] = hi
            X[c, p, q, 1] = lo
        el = np.full((NCORES, P, NCH), -1.0, np.float32)
        el[core_tok, p_tok, q_tok] = slot_of_ent[e_of_tok].astype(np.float32)
        for c in range(NCORES):
            in_maps.append({
                "x": X[c].reshape(P, NCH * 2 * D),
                "el": el[c],
            })
    else:
        X = np.zeros((NCORES, P, NCH, D), np.float32)
        X[core_tok, p_tok, q_tok] = enc_seq
        el = np.full((NCORES, P, NCH), -1.0, np.float32)
        el[core_tok, p_tok, q_tok] = slot_of_ent[e_of_tok].astype(np.float32)
        rw = np.zeros((NCORES, P, NCH), np.float32)
        rw[core_tok, p_tok, q_tok] = w_tok
        for c in range(NCORES):
            in_maps.append({
                "x": X[c].reshape(P, NCH * D),
                "el": el[c],
                "rw": rw[c],
            })

    meta = dict(KPT=KPT, C=C, D=D, E=E, mode=mode,
                core_e=(tile_of_ent // KPT).astype(np.int64),
                jj_e=(tile_of_ent % KPT).astype(np.int64),
                slot_of_ent=slot_of_ent)
    return in_maps, meta


def _unshard(results, meta):
    out_all = np.stack([results[c]["out"] for c in range(NCORES)])
    if "hs_e" in meta:
        # partition-major [8, P, U*D] -> entity (core, u, h*64+s)
        arr = out_all.reshape(NCORES, P, meta["KPT"], meta["D"])
        picked = arr[meta["core_e"], meta["hs_e"], meta["u_e"]]
        return np.ascontiguousarray(picked).astype(np.float32)
    rows = meta["jj_e"] * P + meta["slot_of_ent"]
    return np.ascontiguousarray(out_all[meta["core_e"], rows]).astype(np.float32)


def run(enc_seq, token2mention, mention2entity, num_mentions, num_entities,
        repeat=1, mode="e3m4p", _prog_cache={}):
    """Full pipeline; returns (result, BassKernelResults)."""
    from concourse.bass_utils import run_bass_kernel_spmd

    in_maps, meta = _prepare(enc_seq, token2mention, mention2entity,
                             num_mentions, num_entities, mode=mode)
    key = (meta["KPT"], meta["C"], meta["D"], repeat, mode)
    if key not in _prog_cache:
        _prog_cache[key] = _build_program(meta["KPT"], meta["C"], meta["D"],
                                          repeat=repeat, mode=mode)
    nc = _prog_cache[key]
    res = run_bass_kernel_spmd(nc, in_maps, core_ids=list(range(NCORES)))
    return _unshard(res.results, meta), res


def kernel(enc_seq, token2mention, mention2entity, num_mentions, num_entities):
    result, _ = run(enc_seq, token2mention, mention2entity,
                    num_mentions, num_entities)
    return result



# revision 4
# speedup vs baseline: 1.2352x; 1.2352x over previous
"""Trainium2 Bass kernel v2 for two-level segment mean (tokens->mentions->entities).

Math: collapses to one weighted segment-sum over tokens:
    entities[e] = sum_{t: ent(t)=e} enc_seq[t] * (1/cnt_m[men(t)]) * (1/max(cnt_e[e],1))

Device plan (per core, SPMD over 8 cores):
  - tokens sorted by entity, sequential-packed into tiles of <=1024 tokens
    (8 chunks of 128) x <=64 entity slots; entities split at tile/core
    boundaries (host sums the fragments afterwards).
  - X staged as fp8 e4m3 carrying enc*(w*SC) with host-side ERROR-FEEDBACK
    quantization along each fragment chain (sum of quantized values tracks
    the exact sum to ~half an ulp), rel err ~1e-2 vs 2.7e-2 plain.
  - PE: DoubleRow fp8e4 matmuls contract K=256 (two 128-token chunks per
    instruction) against one-hot slot matrices: ~196 cyc per 256 tokens,
    1.9x the fp8e3 rate.  Two tiles bank-packed per PSUM bank (cols 0:192 /
    192:384), start=True only on the bank's first matmul.
  - one-hot build: DVE batched broadcast is_equal (fp8e4 out) over each
    8-tile group (hits a multi-element/cycle DVE fast path, ~19us/core).
  - PSUM: one 4-bank psum tile per 8-tile group (2 tiles of 64 slots
    bank-packed per bank via start=False accumulation); one ACT copy per
    group moves all 4 banks psum->SBUF (scaled by 1/SC, bf16 out).
  - DMA: X groups of 16 tiles alternate between the two HWDGE queues
    (sync/scalar), ~620 GB/s/core sustained; out DMAs alternate queues too.
  - out [64, NT*D] bf16 per core; host gathers fragments into [E, D] f32.

Measured (repeat-slope, 8 cores, quiet host): ~46-48us vs ~70-76us for the
previous e3m4 one-matmul-per-chunk kernel; rel err ~1.03e-2 (gate 2e-2).
"""

import sys

import numpy as np

for _p in ("/opt/trn_rl_repo",):
    if _p not in sys.path:
        sys.path.insert(0, _p)

P = 128
D = 192
CPT = 8            # 128-token chunks per tile
SLOTS = 64
NCORES = 8
SC = np.float32(32.0)
ACT_PAIRS = 0      # one-hot chunk-pairs per 8-tile group built on ACT (of 32)
GB = 8             # tiles per psum/compute group
DMAG = 16          # tiles per X DMA (2 compute groups)


def _build_program(NT, repeat=1, parts="full", dmag=DMAG, xbufs=4, obufs=6,
                   ogrp=1):
    """SPMD program: NT tiles of CPT chunks, identical for all cores.
    parts: full | dma (X+el+out DMA only) | onehot (dma+onehot) |
    pe (dma+onehot+matmul, no copies/out).  ogrp: compute groups per og
    tile/out-DMA."""
    import concourse.bacc as bacc
    import concourse.mybir as mybir
    import concourse.tile as tile
    from concourse.bass import broadcast_tensor_aps

    f84 = mybir.dt.float8e4
    bf16 = mybir.dt.bfloat16
    f32 = mybir.dt.float32
    NSL = NT * CPT

    nc = bacc.Bacc("TRN2", target_bir_lowering=False, debug=False,
                   num_devices=NCORES)
    x_d = nc.dram_tensor("x", [P, NSL * D], f84, kind="ExternalInput")
    el_d = nc.dram_tensor("el", [P, NSL], bf16, kind="ExternalInput")
    io_d = nc.dram_tensor("io", [P, SLOTS], bf16, kind="ExternalInput")
    out_d = nc.dram_tensor("out", [SLOTS, NT * D], bf16, kind="ExternalOutput")

    with tile.TileContext(nc) as tc:
        with (
            tc.tile_pool(name="const", bufs=1) as const,
            tc.tile_pool(name="x", bufs=xbufs) as xpool,
            tc.tile_pool(name="s", bufs=3) as spool,
            tc.tile_pool(name="a", bufs=2) as apool,
            tc.tile_pool(name="psum", bufs=2, space="PSUM") as ppool,
            tc.tile_pool(name="o", bufs=obufs) as opool,
        ):
            iota_t = const.tile([P, SLOTS], bf16)
            nc.sync.dma_start(out=iota_t[:], in_=io_d[:, :])
            el_sb = const.tile([P, NSL], bf16)
            nc.sync.dma_start(out=el_sb[:], in_=el_d[:, :])

            def body():
                n_dmag = -(-NT // dmag)
                xt_of = {}

                def issue_xdma(dg):
                    if dg >= n_dmag:
                        return
                    jd = dg * dmag
                    dn = min(dmag, NT - jd)
                    xt = xpool.tile([P, dmag * CPT * D], f84, tag="xt")
                    eng = nc.sync if dg % 2 == 0 else nc.scalar
                    eng.dma_start(
                        out=xt[:, :dn * CPT * D],
                        in_=x_d[:, jd * CPT * D:(jd + dn) * CPT * D])
                    for j in range(jd, jd + dn):
                        xt_of[j] = (xt, (j - jd) * CPT * D)

                issue_xdma(0)
                issue_xdma(1)
                issue_xdma(2)

                for jg in range(0, NT, GB):
                    if jg % dmag == 0:
                        issue_xdma(jg // dmag + 3)
                    gn = min(GB, NT - jg)
                    nsl_g = gn * CPT
                    if parts == "dma":
                        og_full = opool.tile([P, GB * D], bf16, tag="og")
                        og = og_full[:SLOTS, :]
                        nc.vector.memset(og[:, :1], 0.0)
                        oeng = nc.scalar if (jg // GB) % 2 == 0 else nc.sync
                        oeng.dma_start(
                            out=out_d[:, jg * D:(jg + gn) * D],
                            in_=og[:, :gn * D])
                        continue
                    # one-hot slices for the whole group
                    sg = spool.tile([P, GB * CPT * SLOTS], f84, tag="sg")
                    n_act = min(ACT_PAIRS * 2, nsl_g) if gn == GB else 0
                    n_dve = nsl_g - n_act
                    i3 = iota_t[:, :].rearrange("p (a c) -> p a c", a=1)
                    e3 = el_sb[:, jg * CPT:jg * CPT + n_dve].rearrange(
                        "p (a c) -> p a c", c=1)
                    ib, eb = broadcast_tensor_aps(i3, e3)
                    nc.vector.tensor_tensor(
                        out=sg[:, :n_dve * SLOTS].rearrange(
                            "p (a c) -> p a c", c=SLOTS),
                        in0=ib, in1=eb, op=mybir.AluOpType.is_equal)
                    if n_act:
                        eln = apool.tile([P, GB * 2 * ACT_PAIRS], bf16,
                                         tag="eln")
                        nc.scalar.activation(
                            out=eln[:, :n_act],
                            in_=el_sb[:, jg * CPT + n_dve:jg * CPT + nsl_g],
                            func=mybir.ActivationFunctionType.Copy,
                            scale=-1.0)
                        ab = apool.tile([P, GB * 2 * ACT_PAIRS * SLOTS], f84,
                                        tag="ab")
                        for k in range(n_act):
                            nc.scalar.activation(
                                out=ab[:, k * SLOTS:(k + 1) * SLOTS],
                                in_=iota_t[:, :],
                                func=mybir.ActivationFunctionType.Abs,
                                bias=eln[:, k:k + 1], scale=1.0)
                            nc.scalar.activation(
                                out=sg[:, (n_dve + k) * SLOTS:
                                       (n_dve + k + 1) * SLOTS],
                                in_=ab[:, k * SLOTS:(k + 1) * SLOTS],
                                func=mybir.ActivationFunctionType.Relu,
                                bias=1.0, scale=-1.0)

                    if parts == "onehot":
                        og_full = opool.tile([P, GB * D], bf16, tag="og")
                        og = og_full[:SLOTS, :]
                        nc.vector.memset(og[:, :1], 0.0)
                        oeng = nc.scalar if (jg // GB) % 2 == 0 else nc.sync
                        oeng.dma_start(
                            out=out_d[:, jg * D:(jg + gn) * D],
                            in_=og[:, :gn * D])
                        continue
                    gi = jg // GB
                    if gi % ogrp == 0:
                        og_full = opool.tile([P, ogrp * GB * D], bf16,
                                             tag="og")
                        og_cur = og_full[:SLOTS, :]
                        og_base = jg
                    og = og_cur
                    ogof = (jg - og_base) * D
                    BANK = 512  # f32 elems per psum bank per partition
                    nbank = (gn + 1) // 2
                    pst = ppool.tile([SLOTS, 4 * BANK], f32, tag="ps")
                    for b in range(nbank):
                        t0 = jg + 2 * b
                        nb = min(2, jg + gn - t0)  # tiles in this bank
                        for pr in range(CPT // 2):
                            for t in range(nb):
                                j = t0 + t
                                xt, xof = xt_of[j]
                                sl = (j - jg) * CPT + pr * 2
                                nc.tensor.matmul(
                                    out=pst[:, b * BANK + t * D:
                                            b * BANK + (t + 1) * D],
                                    lhsT=sg[:, sl * SLOTS:(sl + 2) * SLOTS]
                                    .rearrange("p (k m) -> p k m", k=2),
                                    rhs=xt[:, xof + (pr * 2) * D:
                                           xof + (pr * 2 + 2) * D]
                                    .rearrange("p (k n) -> p k n", k=2),
                                    start=(pr == 0 and t == 0),
                                    stop=(pr == CPT // 2 - 1 and t == nb - 1),
                                    perf_mode=mybir.MatmulPerfMode.DoubleRow,
                                    tile_position=(0, 0),
                                    skip_group_check=True)
                    if parts == "full":
                        nc.scalar.activation(
                            out=og[:, ogof:ogof + nbank * 2 * D].rearrange(
                                "p (a c) -> p a c", c=2 * D),
                            in_=pst[:, :].rearrange(
                                "p (a c) -> p a c", c=BANK)[:, :nbank, :2 * D],
                            func=mybir.ActivationFunctionType.Copy,
                            scale=float(1.0 / SC))
                    if parts == "full" and (gi % ogrp == ogrp - 1
                                            or jg + gn >= NT):
                        oeng = nc.scalar if (gi // ogrp) % 2 == 0 else nc.sync
                        oeng.dma_start(
                            out=out_d[:, og_base * D:(jg + gn) * D],
                            in_=og[:, :(jg + gn - og_base) * D])

            if repeat == 1:
                body()
            else:
                with tc.For_i(0, repeat, 1):
                    body()

    nc.compile()
    return nc


def _pack(e_of_tok, T):
    """Sequential entity packing. Returns per-token (core, slice, part, slot),
    fragment table, and NT."""
    order = np.argsort(e_of_tok, kind="stable")
    es = e_of_tok[order]
    new_ent = np.empty(T, bool)
    new_ent[0] = True
    np.not_equal(es[1:], es[:-1], out=new_ent[1:])
    rs = np.flatnonzero(new_ent)          # run starts (global sorted idx)

    TPC = -(-T // NCORES)
    cuts_per_core = []
    NT = 0
    for c in range(NCORES):
        lo, hi = c * TPC, min((c + 1) * TPC, T)
        cuts = [lo]
        pos = lo
        while pos < hi:
            cap = min(pos + CPT * P, hi)
            j0 = np.searchsorted(rs, pos + 1)
            j1 = np.searchsorted(rs, cap)
            n_ent = 1 + (j1 - j0)
            if n_ent <= SLOTS:
                pos = cap
            else:
                pos = int(rs[j0 + SLOTS - 1])
            cuts.append(pos)
        cuts_per_core.append(np.asarray(cuts, np.int64))
        NT = max(NT, len(cuts) - 1)

    # per-token coordinates
    core_s = np.empty(T, np.int32)
    tile_s = np.empty(T, np.int32)
    local_s = np.empty(T, np.int64)
    tile_start_s = np.empty(T, np.int64)
    for c in range(NCORES):
        cuts = cuts_per_core[c]
        lo, hi = cuts[0], cuts[-1]
        idx = np.arange(lo, hi)
        t_of = np.searchsorted(cuts, idx, side="right") - 1
        core_s[lo:hi] = c
        tile_s[lo:hi] = t_of
        tile_start_s[lo:hi] = cuts[t_of]
        local_s[lo:hi] = idx - cuts[t_of]

    # fragments: new entity OR tile start
    is_tile_start = np.zeros(T, bool)
    for c in range(NCORES):
        cs = cuts_per_core[c][:-1]
        is_tile_start[cs[cs < T]] = True
    frag_b = new_ent | is_tile_start
    frag_id = np.cumsum(frag_b) - 1
    # slot = frag rank within tile
    frag_at_tile_start = frag_id[tile_start_s]
    slot_s = (frag_id - frag_at_tile_start).astype(np.int32)
    assert slot_s.max() < SLOTS

    slice_s = (tile_s.astype(np.int64) * CPT + local_s // P).astype(np.int32)
    part_s = (local_s % P).astype(np.int32)

    # fragment table: entity, core, tile, slot (first token of each frag)
    fs = np.flatnonzero(frag_b)
    frag_ent = es[fs]
    frag_core = core_s[fs]
    frag_tile = tile_s[fs]
    frag_slot = slot_s[fs]

    return (order, es, frag_b, core_s, slice_s, part_s, slot_s,
            (frag_ent, frag_core, frag_tile, frag_slot), NT)


def _quantize_feedback(v_sorted, frag_b):
    """Error-feedback e4m3 quantization along fragment chains.
    v_sorted: [T, d] f32 (sorted order). Returns e4m3 array [T, d]."""
    import ml_dtypes
    T = v_sorted.shape[0]
    frag_id = np.cumsum(frag_b) - 1
    nfrag = int(frag_id[-1]) + 1
    starts = np.flatnonzero(frag_b)
    lens = np.diff(np.append(starts, T))
    maxlen = int(lens.max())
    q8 = np.empty(v_sorted.shape, ml_dtypes.float8_e4m3)
    c = np.zeros((nfrag, v_sorted.shape[1]), np.float32)
    active = np.arange(nfrag)
    for k in range(maxlen):
        if k > 0:
            active = active[lens[active] > k]
        idx = starts[active] + k
        v = v_sorted[idx] + c[active]
        np.clip(v, -224.0, 224.0, out=v)
        q = v.astype(ml_dtypes.float8_e4m3)
        q8[idx] = q
        c[active] = v - q.astype(np.float32)
    return q8


def _prepare(enc_seq, token2mention, mention2entity, num_mentions,
             num_entities):
    import ml_dtypes
    enc_seq = np.ascontiguousarray(np.asarray(enc_seq, dtype=np.float32))
    t2m = np.asarray(token2mention).astype(np.int64, copy=False)
    m2e = np.asarray(mention2entity).astype(np.int64, copy=False)
    M = int(num_mentions)
    E = int(num_entities)
    T, Din = enc_seq.shape
    assert Din == D

    e_of_tok = m2e[t2m]
    cnt_m = np.bincount(t2m, minlength=M)
    cnt_e = np.bincount(m2e, minlength=E)
    w_tok = ((1.0 / np.maximum(cnt_m, 1))[t2m]
             * (1.0 / np.maximum(cnt_e, 1))[e_of_tok]).astype(np.float32)

    (order, es, frag_b, core_s, slice_s, part_s, slot_s, frags,
     NT) = _pack(e_of_tok, T)

    NSL = NT * CPT
    X = np.zeros((NCORES, P, NSL, D), ml_dtypes.float8_e4m3)
    el = np.full((NCORES, P, NSL), -1.0, ml_dtypes.bfloat16)
    el[core_s, part_s, slice_s] = slot_s.astype(ml_dtypes.bfloat16)

    ws = (w_tok * SC)[order]
    SLAB = 64
    for d0 in range(0, D, SLAB):
        d1 = min(d0 + SLAB, D)
        v = enc_seq[:, d0:d1][order] * ws[:, None]
        q8 = _quantize_feedback(v, frag_b)
        X[core_s, part_s, slice_s, d0:d1] = q8

    io = np.broadcast_to(np.arange(SLOTS, dtype=np.float32),
                         (P, SLOTS)).astype(ml_dtypes.bfloat16)
    in_maps = []
    for c in range(NCORES):
        in_maps.append({
            "x": X[c].reshape(P, NSL * D),
            "el": el[c],
            "io": np.ascontiguousarray(io),
        })
    meta = dict(NT=NT, E=E, frags=frags)
    return in_maps, meta


def _unshard(results, meta):
    E = meta["E"]
    NT = meta["NT"]
    frag_ent, frag_core, frag_tile, frag_slot = meta["frags"]
    outs = np.stack([np.asarray(results[c]["out"]).astype(np.float32)
                     for c in range(NCORES)])           # [8, 64, NT*D]
    outs = outs.reshape(NCORES, SLOTS, NT, D)
    vals = outs[frag_core, frag_slot, frag_tile]        # [nfrag, D]
    res = np.zeros((E, D), np.float32)
    np.add.at(res, frag_ent, vals)
    return res


def run(enc_seq, token2mention, mention2entity, num_mentions, num_entities,
        repeat=1, _prog_cache={}):
    from concourse.bass_utils import run_bass_kernel_spmd

    in_maps, meta = _prepare(enc_seq, token2mention, mention2entity,
                             num_mentions, num_entities)
    key = (meta["NT"], repeat)
    if key not in _prog_cache:
        _prog_cache[key] = _build_program(meta["NT"], repeat=repeat)
    nc = _prog_cache[key]
    res = run_bass_kernel_spmd(nc, in_maps, core_ids=list(range(NCORES)))
    return _unshard(res.results, meta), res


def kernel(enc_seq, token2mention, mention2entity, num_mentions, num_entities):
    """Graded entry point.  Retries the device run on implausible output
    (NaN/Inf or wildly out-of-range values) to guard against rare first-run
    execution flakes observed on shared hardware."""
    from concourse.bass_utils import run_bass_kernel_spmd

    in_maps, meta = _prepare(enc_seq, token2mention, mention2entity,
                             num_mentions, num_entities)
    nc = _build_program(meta["NT"], repeat=1)
    for attempt in range(3):
        res = run_bass_kernel_spmd(nc, in_maps, core_ids=list(range(NCORES)))
        result = _unshard(res.results, meta)
        if np.isfinite(result).all() and np.abs(result).max() < 1e3:
            return result
    return result
